# revision 46
# baseline (speedup 1.0000x reference)
"""Trainium2 Bass kernel for chunked flash-attention block (B=2, S=2048, D=1024, H=16).

Sharding: 8 cores = 2 batches x 4 head-groups (4 heads each). Each core computes
its heads' QKV projections + RoPE + per-chunk-softmax attention + its slice of the
output projection; the host sums the 4 partial out-projections per batch.

Schedule (cost-model driven):
- Scores run as fp8 DoubleRow matmuls (0.5 cycles/row): k single-quantized to
  fp8e4, q stored as an fp8 (hi, lo) pair so q keeps ~bf16 precision; only the
  k-side quantization adds noise (damped by the small score magnitudes).
- exp is the ScalarE bottleneck; 1-2 score slots per (hp, c, n) group are
  computed on DVE instead via the Schraudolph bit trick
  (i32 = trunc(A*s + B); bitcast f32), keeping ACT ~= DVE ~= PE busy.
- xT is DMA'd in 512-column blocks interleaved with the small weight tensors so
  the first score batch fires early; W@V accumulates t2-outer so it streams
  behind the exps; the out-projection is emitted in m-tile pairs fused into the
  last four groups, with an interleaved drain for the final group.
- GpSimd (Pool) takes only SBUF->SBUF chain-tail ops (q hi/lo split, k dup,
  exp bitcast copies); it cannot read PSUM on this backend and is ~4x slower
  per element, so everything latency-critical stays on DVE.
"""

import os

import numpy as np
import ml_dtypes

import concourse.bass as bass
import concourse.tile as tile
from concourse import bacc, mybir
from concourse.bass_utils import run_bass_kernel_spmd
from concourse.masks import make_identity

dt = mybir.dt
F32 = dt.float32
BF16 = dt.bfloat16
FP8 = dt.float8e4
I32 = dt.int32
AF = mybir.ActivationFunctionType
OP = mybir.AluOpType
DR = mybir.MatmulPerfMode.DoubleRow

B, S, D, H, HD = 2, 2048, 1024, 16, 64
CHUNK = 1024
NHL = 4              # local heads per core
JL = NHL * HD        # 256 local projected dims
ND = D // 128        # 8 k-tiles for the projections
NSK = S // 128       # 16 sk p-tiles
TPC = CHUNK // 128   # 8 sk tiles per chunk
NM = S // 128        # 16 sq p-tiles

# Schraudolph exp: exp(x) ~= bitcast_f32(trunc(A*x + BIAS))
EXP_A = float(2**23 / np.log(2))
EXP_BIAS = float(127 * 2**23 - 486411)

SC_BATCHES0 = ((0, 3), (3, 3), (6, 3), (9, 3), (12, 3), (15, 1))  # all on ACT
SC_BATCHES1 = ((1, 3), (4, 3), (7, 3), (10, 3), (13, 3))  # slot 0 offloaded
SC_BATCHES2 = ((2, 3), (5, 3), (8, 3), (11, 3), (14, 2))  # slots 0,1 offloaded

_CACHED = {}


def _emit_body(nc, tc, persist, rope, aps, rep):
    """Emit one full iteration of the kernel into the open TileContext."""
    xT_d, wq_d, wk_d, wv_d, wo_d, c2_d, s2_d, out_d = aps
    r = f"r{rep}"

    # ---------------- on-device constants (no DMA dependency) ---------------
    ident = persist.tile([128, 128], BF16, tag="ident", name=f"ident_{r}")
    make_identity(nc, ident[:])
    wident = persist.tile([128, 512], BF16, tag="wident", name=f"wident_{r}")
    nc.gpsimd.memset(wident[:], 0.0)

    # ---------------- DMA order (transfers serialize in the DMA model) ------
    wsbs = {}
    wsbs["wq"] = persist.tile([128, ND, JL], BF16, tag="wq", name=f"wq_{r}")
    nc.sync.dma_start(wsbs["wq"][:], wq_d.rearrange("(t p) j -> p t j", p=128))
    xsb = persist.tile([128, ND, S], BF16, tag="xT", name=f"xT_{r}")
    xT_r = xT_d.rearrange("(t p) s -> p t s", p=128)
    c2 = persist.tile([128, S], BF16, tag="c2", name=f"c2_{r}")
    s2 = persist.tile([128, S], BF16, tag="s2", name=f"s2_{r}")

    def dma_xblock(st):
        # K_XDMA=1|2|8: DMAs per st-block of xT. Fewer DMAs amortize the
        # ~650ns HWDGE+DGE cadence; more give finer per-di matmul streaming.
        sl = slice(st * 512, (st + 1) * 512)
        nx = int(os.environ.get("K_XDMA", "4"))
        step = ND // nx
        for dh in range(nx):
            dsl = slice(dh * step, (dh + 1) * step)
            nc.sync.dma_start(xsb[:, dsl, sl], xT_r[:, dsl, sl])
        nc.sync.dma_start(c2[:, sl], c2_d[:, sl])
        nc.sync.dma_start(s2[:, sl], s2_d[:, sl])

    dma_xblock(0)
    wsbs["wk"] = persist.tile([128, ND, JL], BF16, tag="wk", name=f"wk_{r}")
    nc.sync.dma_start(wsbs["wk"][:], wk_d.rearrange("(t p) j -> p t j", p=128))
    dma_xblock(1)
    wsbs["wv"] = persist.tile([128, ND, JL], BF16, tag="wv", name=f"wv_{r}")
    nc.sync.dma_start(wsbs["wv"][:], wv_d.rearrange("(t p) j -> p t j", p=128))
    dma_xblock(2)
    dma_xblock(3)
    wo_sb = persist.tile([128, 2, D], BF16, tag="wo", name=f"wo_{r}")
    nc.sync.dma_start(wo_sb[:], wo_d.rearrange("(t p) n -> p t n", p=128))

    # ---------------- persistent activations -------------------------------
    # q: (hi, lo) fp8 pair; k: fp8 duplicated along dim2 for the DoubleRow
    # trick  scores = (q_hi + q_lo)^T k  =  DR(lhsT=(k,k), rhs=(q_hi,q_lo)).
    qT8s = [persist.tile([128, 2, 2, 512], FP8, tag=f"qT8_{st}", name=f"qT8{st}_{r}")
            for st in range(4)]
    kT8s = [persist.tile([128, 2, 2, 512], FP8, tag=f"kT8_{st}", name=f"kT8{st}_{r}")
            for st in range(4)]
    vON = persist.tile([128, NSK, NHL * 65], BF16, tag="vON", name=f"vON_{r}")
    attn = persist.tile([128, NM, JL], BF16, tag="attn", name=f"attn_{r}")
    attnT = persist.tile([128, 2, S], BF16, tag="attnT", name=f"attnT_{r}")

    vON_r = vON[:].rearrange("p t (h c) -> p (t h) c", c=65)
    nc.vector.memset(vON_r[:, :, 64:65], 1.0)

    def emit_qk_tile(pool, w, jt, st, fast=False, as_steps=False):
        """One [128, 512] q/k projection tile + RoPE into qT8/kT8.
        fast=True keeps the fp8 chain-tail ops on DVE (lead-in latency);
        otherwise they ride on GpSimd. as_steps returns a list of emission
        closures (projection first) for interleaving with another tile."""
        sl = slice(st * 512, (st + 1) * 512)

        def s_proj():
            ps = pool.tile([128, 512], F32, tag="ps1", name=f"pqk_{r}")
            for di in range(ND):
                nc.tensor.matmul(
                    ps[:],
                    lhsT=wsbs[w][:, di, jt * 128:(jt + 1) * 128],
                    rhs=xsb[:, di, sl],
                    start=(di == 0),
                    stop=(di == ND - 1),
                )
            st_state["ps"] = ps

        st_state = {}

        def s_qb():
            if fast and os.environ.get("K_FASTPS", "0") == "1":
                return  # muls read the PSUM directly; skip the staging copy
            qb = rope.tile([128, 512], BF16, tag="qb", name=f"qb_{r}")
            nc.vector.tensor_copy(qb[:], st_state["ps"][:])
            st_state["qb"] = qb

        def s_muls():
            qb = st_state.get("qb")
            src_ap = st_state["ps"][:] if qb is None else qb[:]
            w2 = rope.tile([128, 512], BF16, tag="w2", name=f"w2_{r}")
            nc.vector.tensor_mul(w2[:], src_ap, s2[:, sl])
            t2 = rope.tile([128, 512], BF16, tag="t2", name=f"t2_{r}")
            nc.vector.tensor_mul(t2[:], src_ap, c2[:, sl])
            st_state["w2"], st_state["t2"] = w2, t2

        def s_swap():
            w2 = st_state["w2"]
            u = rope.tile([128, 512], BF16, tag="u", name=f"u_{r}")
            for blk in range(4):
                o = blk * 32
                so = o ^ 32
                nc.vector.tensor_copy(u[o:o + 32, :], w2[so:so + 32, :])
            st_state["u"] = u

        def s_tail():
            t2, u = st_state["t2"], st_state["u"]
            use_pool = (not fast) and os.environ.get("K_QPOOL", "1") == "1"
            eng = nc.gpsimd if use_pool else nc.vector
            if w == "wk":
                kT8 = kT8s[st]
                nc.vector.tensor_sub(kT8[:, jt, 0, :], t2[:], u[:])
                nc.vector.tensor_copy(kT8[:, jt, 1, :], kT8[:, jt, 0, :])
            else:
                qT8 = qT8s[st]
                qf = rope.tile([128, 512], BF16, tag="qf", name=f"qf_{r}")
                nc.vector.tensor_sub(qf[:], t2[:], u[:])
                eng.tensor_copy(qT8[:, jt, 0, :], qf[:])
                eng.tensor_sub(qT8[:, jt, 1, :], qf[:], qT8[:, jt, 0, :])

        steps = [s_proj, s_qb, s_muls, s_swap, s_tail]
        if as_steps:
            return steps
        for f in steps:
            f()

    def emit_qk_pair(pool, spec_a, spec_b, fast=False):
        """Two projection tiles with their rope chains interleaved on DVE so
        chain-internal latencies overlap."""
        steps = [emit_qk_tile(pool, w, jt, st, fast=fast, as_steps=True)
                 for (w, jt, st) in (spec_a, spec_b)]
        for pair in zip(*steps):
            for f in pair:
                f()

    def emit_v_tile(pool, st):
        psv = pool.tile([128, JL], F32, tag="ps1", name=f"pv_{r}")
        for di in range(ND):
            nc.tensor.matmul(
                psv[:],
                lhsT=xsb[:, di, st * 128:(st + 1) * 128],
                rhs=wsbs["wv"][:, di, :],
                start=(di == 0),
                stop=(di == ND - 1),
            )
        nc.vector.tensor_copy(
            vON_r[:, st * NHL:(st + 1) * NHL, 0:64],
            psv[:].rearrange("p (h e) -> p h e", e=64),
        )

    # ---------------- main pipeline ----------------------------------------
    with (
        tc.tile_pool(name=f"sc_{r}", bufs=2, space="PSUM") as scp,
        tc.tile_pool(name=f"ps1_{r}", bufs=2, space="PSUM") as ps1p,
        tc.tile_pool(name=f"expp_{r}", bufs=int(os.environ.get("K_EXPB", "3"))) as expp,
        tc.tile_pool(name=f"normp_{r}", bufs=8) as normp,
        tc.tile_pool(name=f"osb_{r}", bufs=8) as osbp,
    ):
        # PE warm-up on the on-device zero tile: spans the xT DMA window so the
        # p-state ramp is done (and PE busy) when the first projections land.
        for i in range(int(os.environ.get("K_WARM", "12"))):
            warm = ps1p.tile([128, 512], F32, tag="ps1", name=f"warm_{r}")
            nc.tensor.matmul(warm[:], lhsT=ident[:], rhs=wident[:],
                             start=True, stop=True)
        # prefetch the ScalarE exp activation table (~1.3us).
        twarm = normp.tile([128, 2], F32, tag="rec", name=f"twarm_{r}")
        nc.scalar.activation(out=twarm[:, :], in_=ident[:, 0:2], func=AF.Exp)

        def score_mm(out_ap, hp, h, tg, n):
            p0 = 64 * h
            kst, ko = tg // 4, tg % 4
            nc.tensor.matmul(
                out_ap,
                lhsT=kT8s[kst][p0:p0 + 64, hp, :, ko * 128:(ko + 1) * 128],
                rhs=qT8s[n][p0:p0 + 64, hp, :, :],
                start=True, stop=True,
                perf_mode=DR,
            )

        def emit_scd(hp, c, n, exp_tiles, nd):
            """Offloaded score slots at slot top: their PSUM tiles live in the
            fast-recycling ps1 rotation, so the DVE/Pool exps land early in the
            group instead of gating anything. Slots 0(,1) = t2=0, whose k tile
            is the first of the chunk (widest producer slack)."""
            et = expp.tile([128, 16, 512], BF16, tag="expT", name=f"expT_{r}")
            exp_tiles[n] = et
            for slot in range(nd):
                scd = ps1p.tile([128, 512], F32, tag="ps1", name=f"scd_{r}")
                score_mm(scd[:], hp, slot % 2, c * TPC + slot // 2, n)
                i32 = rope.tile([128, 512], I32, tag="i32", name=f"i32_{r}")
                nc.vector.tensor_scalar(i32[:], scd[:], EXP_A, EXP_BIAS,
                                        OP.mult, OP.add)
                nc.gpsimd.tensor_copy(et[:, slot, :], i32[:].bitcast(F32))

        def emit_scores(hp, c, n, exp_tiles, nd, part=None, bsel=None):
            et = exp_tiles[n]
            batches = (SC_BATCHES0, SC_BATCHES1, SC_BATCHES2)[nd]
            sp = int(os.environ.get("K_SPLIT", "2"))
            if bsel is not None:
                batches = batches[bsel]
            elif part == 0:
                batches = batches[:sp]
            elif part == 1:
                batches = batches[sp:]
            for start, size in batches:
                sc = scp.tile([128, size, 512], F32, tag="sc", name=f"sc_{r}")
                for k in range(size):
                    slot = start + k
                    t2_, h = slot // 2, slot % 2
                    score_mm(sc[:, k, :], hp, h, c * TPC + t2_, n)
                nc.scalar.activation(
                    out=et[:, start:start + size, :],
                    in_=sc[:, 0:size, :],
                    func=AF.Exp,
                )

        def wv_mms(psB, et, hp, c, m2, t2_):
            tg = c * TPC + t2_
            for j in range(2):
                for h in range(2):
                    hl = hp * 2 + h
                    off = j * 132 + h * 66
                    nc.tensor.matmul(
                        psB[:, off:off + 65],
                        lhsT=et[:, 2 * t2_ + h, (m2 + j) * 128:(m2 + j + 1) * 128],
                        rhs=vON[:, tg, hl * 65:(hl + 1) * 65],
                        start=(t2_ == 0 and j == 0 and h == 0),
                        stop=(t2_ == TPC - 1 and j == 1 and h == 1),
                        skip_group_check=True,
                    )

        def wv_norm(psB, hp, c, n, m2):
            psBr = psB[:].rearrange("p (j h e) -> p j h e", h=2, e=66)
            rec = normp.tile([128, 2, 2], F32, tag="rec", name=f"rec_{r}")
            nc.vector.reciprocal(rec[:, :, :], psBr[:, :, :, 64:65])
            recb = rec[:].unsqueeze(3).broadcast_to([128, 2, 2, 64])
            m0 = n * 4 + m2
            attn_v = attn[:, m0:m0 + 2, hp * 128:(hp + 1) * 128].rearrange(
                "p j (h e) -> p j h e", e=64)
            if c == 0:
                nc.vector.tensor_tensor(out=attn_v, in0=psBr[:, :, :, 0:64],
                                        in1=recb, op=OP.mult)
            else:
                ntmp = normp.tile([128, 2, 2, 64], BF16, tag="ntmp", name=f"nt_{r}")
                nc.vector.tensor_tensor(out=ntmp[:], in0=psBr[:, :, :, 0:64],
                                        in1=recb, op=OP.mult)
                nc.vector.tensor_add(attn_v, attn_v, ntmp[:])

        def emit_wv_double(hp, c, n, exp_tiles):
            """Both W@V halves of a group with interleaved t2 passes. Runs a
            full slot after the group's scores, so every exp is (nearly) done
            and the psB accumulators live only briefly — keeping the shared
            ps1 PSUM rotation free of long-straddling tiles."""
            et = exp_tiles.pop(n)
            psB0 = ps1p.tile([128, 264], F32, tag="ps1", name=f"psB_{r}")
            psB1 = ps1p.tile([128, 264], F32, tag="ps1", name=f"psB_{r}")
            for t2_ in range(TPC):
                wv_mms(psB0, et, hp, c, 0, t2_)
                wv_mms(psB1, et, hp, c, 2, t2_)
            wv_norm(psB0, hp, c, n, 0)
            wv_norm(psB1, hp, c, n, 2)

        def emit_out_pair(mpair, tail=False):
            """Transpose + output projection for two finalized m-tiles."""
            for i, m in enumerate(mpair):
                pt = ps1p.tile([128, 2, 128], BF16, tag="ps1", name=f"pt_{r}")
                for jt in range(2):
                    nc.tensor.matmul(
                        pt[:, jt, :],
                        lhsT=attn[:, m, jt * 128:(jt + 1) * 128], rhs=ident[:],
                        is_transpose=True, start=(jt == 0), stop=(jt == 1),
                        skip_group_check=True)
                dst = attnT[:, :, m * 128:(m + 1) * 128]
                if tail and i % 2 == 1:
                    nc.scalar.copy(dst, pt[:])
                else:
                    nc.vector.tensor_copy(dst, pt[:])
            for i, m in enumerate(mpair):
                for nn in range(2):
                    pso = ps1p.tile([128, 512], F32, tag="ps1", name=f"po_{r}")
                    for jt in range(2):
                        nc.tensor.matmul(
                            pso[:],
                            lhsT=attnT[:, jt, m * 128:(m + 1) * 128],
                            rhs=wo_sb[:, jt, nn * 512:(nn + 1) * 512],
                            start=(jt == 0),
                            stop=(jt == 1),
                        )
                    osb = osbp.tile([128, 512], BF16, tag="osb", name=f"osb_{r}")
                    if tail and (2 * i + nn) % 2 == 1:
                        nc.scalar.copy(osb[:], pso[:])
                    else:
                        nc.vector.tensor_copy(osb[:], pso[:])
                    nc.sync.dma_start(out_d[m * 128:(m + 1) * 128, nn * 512:(nn + 1) * 512], osb[:])

        def qk(w, jt, st, fast=False):
            return lambda: emit_qk_tile(ps1p, w, jt, st, fast=fast)

        def vt(st):
            return lambda: emit_v_tile(ps1p, st)

        # Emission-order constraints (PE queues are in-order, so a consumer
        # emitted before its producer deadlocks): q(0,s) before slot s+1's
        # scores; k(0,2..3) before slot 5; q/k(1,*) before their hp=1 slots;
        # all v tiles of a chunk before that chunk's first wv half.
        fills = {
            0: {  # measured 175.0us
                1: [vt(0), vt(1), vt(2), vt(3), vt(4), vt(5), vt(6), vt(7),
                    qk("wq", 0, 1)],
                2: [qk("wq", 0, 2), qk("wq", 0, 3), vt(8), vt(9)],
                3: [qk("wk", 0, 2), vt(10), vt(11)],
                4: [qk("wk", 0, 3), vt(12), vt(13)],
                5: [vt(14), vt(15)],
                6: [qk("wk", 1, 0), qk("wk", 1, 1)],
                7: [qk("wq", 1, 0)],
                8: [qk("wq", 1, 1)],
                9: [qk("wq", 1, 2)],
                10: [qk("wq", 1, 3), qk("wk", 1, 2)],
                11: [qk("wk", 1, 3)],
            },
            4: {  # k02 pulled into the long slot 1
                1: [vt(0), vt(1), vt(2), vt(3), vt(4), vt(5), vt(6), vt(7),
                    qk("wq", 0, 1), qk("wq", 0, 2), qk("wk", 0, 2)],
                2: [qk("wq", 0, 3), qk("wk", 0, 3)],
                4: [vt(8), vt(9), vt(10), vt(11)],
                5: [vt(12), vt(13), vt(14), vt(15)],
                6: [qk("wk", 1, 0), qk("wk", 1, 1)],
                7: [qk("wq", 1, 0)],
                8: [qk("wq", 1, 1)],
                9: [qk("wq", 1, 2), qk("wk", 1, 2)],
                10: [qk("wq", 1, 3), qk("wk", 1, 3)],
            },
            5: {  # k02+q03 in slot 1, k03 alone in 2
                1: [vt(0), vt(1), vt(2), vt(3), vt(4), vt(5), vt(6), vt(7),
                    qk("wq", 0, 1), qk("wq", 0, 2), qk("wk", 0, 2), qk("wq", 0, 3)],
                2: [qk("wk", 0, 3)],
                4: [vt(8), vt(9), vt(10), vt(11)],
                5: [vt(12), vt(13), vt(14), vt(15)],
                6: [qk("wk", 1, 0), qk("wk", 1, 1)],
                7: [qk("wq", 1, 0)],
                8: [qk("wq", 1, 1)],
                9: [qk("wq", 1, 2), qk("wk", 1, 2)],
                10: [qk("wq", 1, 3), qk("wk", 1, 3)],
            },
            6: {  # everything rope-heavy in slot 1
                1: [vt(0), vt(1), vt(2), vt(3), vt(4), vt(5), vt(6), vt(7),
                    qk("wq", 0, 1), qk("wq", 0, 2), qk("wk", 0, 2), qk("wq", 0, 3),
                    qk("wk", 0, 3)],
                4: [vt(8), vt(9), vt(10), vt(11)],
                5: [vt(12), vt(13), vt(14), vt(15)],
                6: [qk("wk", 1, 0), qk("wk", 1, 1)],
                7: [qk("wq", 1, 0)],
                8: [qk("wq", 1, 1)],
                9: [qk("wq", 1, 2), qk("wk", 1, 2)],
                10: [qk("wq", 1, 3), qk("wk", 1, 3)],
            },
            7: {  # hp1 ropes one slot earlier
                1: [vt(0), vt(1), vt(2), vt(3), vt(4), vt(5), vt(6), vt(7),
                    qk("wq", 0, 1), qk("wq", 0, 2), qk("wk", 0, 2), qk("wq", 0, 3),
                    qk("wk", 0, 3)],
                4: [vt(8), vt(9), vt(10), vt(11)],
                5: [vt(12), vt(13), vt(14), vt(15)],
                6: [qk("wk", 1, 0), qk("wk", 1, 1), qk("wq", 1, 0)],
                7: [qk("wq", 1, 1)],
                8: [qk("wq", 1, 2)],
                9: [qk("wq", 1, 3), qk("wk", 1, 2)],
                10: [qk("wk", 1, 3)],
            },
            8: {  # v c1 tiles earlier, hp1 ropes spread
                1: [vt(0), vt(1), vt(2), vt(3), vt(4), vt(5), vt(6), vt(7),
                    qk("wq", 0, 1), qk("wq", 0, 2), qk("wk", 0, 2), qk("wq", 0, 3),
                    qk("wk", 0, 3)],
                3: [vt(8), vt(9), vt(10), vt(11)],
                4: [vt(12), vt(13), vt(14), vt(15)],
                6: [qk("wk", 1, 0), qk("wk", 1, 1)],
                7: [qk("wq", 1, 0)],
                8: [qk("wq", 1, 1)],
                9: [qk("wq", 1, 2), qk("wk", 1, 2)],
                10: [qk("wq", 1, 3), qk("wk", 1, 3)],
            },
            13: {  # fill9 + hp1 q splits on DVE (Pool tail gates their scds)
                1: [qk("wq", 0, 1), qk("wq", 0, 2), qk("wk", 0, 2), qk("wq", 0, 3),
                    qk("wk", 0, 3), vt(0), vt(1), vt(2), vt(3), vt(4), vt(5),
                    vt(6), vt(7)],
                4: [vt(8), vt(9), vt(10), vt(11)],
                5: [vt(12), vt(13), vt(14), vt(15)],
                6: [qk("wk", 1, 0), qk("wk", 1, 1)],
                7: [qk("wq", 1, 0)],
                8: [qk("wq", 1, 1, True)],
                9: [qk("wq", 1, 2, True), qk("wk", 1, 2)],
                10: [qk("wq", 1, 3, True), qk("wk", 1, 3)],
            },
            9: {  # qk chains first (q01 gates G2), v wall last
                1: [qk("wq", 0, 1), qk("wq", 0, 2), qk("wk", 0, 2), qk("wq", 0, 3),
                    qk("wk", 0, 3), vt(0), vt(1), vt(2), vt(3), vt(4), vt(5),
                    vt(6), vt(7)],
                4: [vt(8), vt(9), vt(10), vt(11)],
                5: [vt(12), vt(13), vt(14), vt(15)],
                6: [qk("wk", 1, 0), qk("wk", 1, 1)],
                7: [qk("wq", 1, 0)],
                8: [qk("wq", 1, 1)],
                9: [qk("wq", 1, 2), qk("wk", 1, 2)],
                10: [qk("wq", 1, 3), qk("wk", 1, 3)],
            },
            10: {  # hp1 ropes one slot earlier
                1: [qk("wq", 0, 1), qk("wq", 0, 2), qk("wk", 0, 2), qk("wq", 0, 3),
                    qk("wk", 0, 3), vt(0), vt(1), vt(2), vt(3), vt(4), vt(5),
                    vt(6), vt(7)],
                4: [vt(8), vt(9), vt(10), vt(11)],
                5: [vt(12), vt(13), vt(14), vt(15)],
                6: [qk("wk", 1, 0), qk("wk", 1, 1), qk("wq", 1, 0)],
                7: [qk("wq", 1, 1), qk("wq", 1, 2)],
                8: [qk("wk", 1, 2), qk("wq", 1, 3)],
                9: [qk("wk", 1, 3)],
            },
            11: {  # only q13 one slot earlier
                1: [qk("wq", 0, 1), qk("wq", 0, 2), qk("wk", 0, 2), qk("wq", 0, 3),
                    qk("wk", 0, 3), vt(0), vt(1), vt(2), vt(3), vt(4), vt(5),
                    vt(6), vt(7)],
                4: [vt(8), vt(9), vt(10), vt(11)],
                5: [vt(12), vt(13), vt(14), vt(15)],
                6: [qk("wk", 1, 0), qk("wk", 1, 1)],
                7: [qk("wq", 1, 0)],
                8: [qk("wq", 1, 1)],
                9: [qk("wq", 1, 2), qk("wk", 1, 2), qk("wq", 1, 3)],
                10: [qk("wk", 1, 3)],
            },
            12: {  # only q10 one slot earlier
                1: [qk("wq", 0, 1), qk("wq", 0, 2), qk("wk", 0, 2), qk("wq", 0, 3),
                    qk("wk", 0, 3), vt(0), vt(1), vt(2), vt(3), vt(4), vt(5),
                    vt(6), vt(7)],
                4: [vt(8), vt(9), vt(10), vt(11)],
                5: [vt(12), vt(13), vt(14), vt(15)],
                6: [qk("wk", 1, 0), qk("wk", 1, 1), qk("wq", 1, 0)],
                7: [qk("wq", 1, 1)],
                8: [qk("wq", 1, 2)],
                9: [qk("wq", 1, 3), qk("wk", 1, 2)],
                10: [qk("wk", 1, 3)],
            },
            3: {  # custom G1 lead-in handles q01/q02/v0-7
                2: [qk("wq", 0, 3), qk("wk", 0, 2)],
                3: [qk("wk", 0, 3)],
                4: [vt(8), vt(9), vt(10), vt(11)],
                5: [vt(12), vt(13), vt(14), vt(15)],
                6: [qk("wk", 1, 0), qk("wk", 1, 1)],
                7: [qk("wq", 1, 0)],
                8: [qk("wq", 1, 1)],
                9: [qk("wq", 1, 2), qk("wk", 1, 2)],
                10: [qk("wq", 1, 3), qk("wk", 1, 3)],
            },
            2: {  # ropes early where DVE is light; v c1 copies late
                1: [vt(0), vt(1), vt(2), vt(3), vt(4), vt(5), vt(6), vt(7),
                    qk("wq", 0, 1), qk("wq", 0, 2)],
                2: [qk("wq", 0, 3), qk("wk", 0, 2)],
                3: [qk("wk", 0, 3)],
                4: [vt(8), vt(9), vt(10), vt(11)],
                5: [vt(12), vt(13), vt(14), vt(15)],
                6: [qk("wk", 1, 0), qk("wk", 1, 1)],
                7: [qk("wq", 1, 0)],
                8: [qk("wq", 1, 1)],
                9: [qk("wq", 1, 2), qk("wk", 1, 2)],
                10: [qk("wq", 1, 3), qk("wk", 1, 3)],
            },
            1: {  # k-tiles one slot earlier
                1: [vt(0), vt(1), vt(2), vt(3), vt(4), vt(5), vt(6), vt(7),
                    qk("wq", 0, 1)],
                2: [qk("wk", 0, 2), qk("wq", 0, 2), vt(8), vt(9)],
                3: [qk("wk", 0, 3), qk("wq", 0, 3), vt(10), vt(11)],
                4: [vt(12), vt(13)],
                5: [vt(14), vt(15)],
                6: [qk("wk", 1, 0), qk("wk", 1, 1)],
                7: [qk("wq", 1, 0)],
                8: [qk("wq", 1, 1)],
                9: [qk("wq", 1, 2)],
                10: [qk("wq", 1, 3), qk("wk", 1, 2)],
                11: [qk("wk", 1, 3)],
            },
        }
        fill = fills[int(os.environ.get("K_FILL", "9"))]

        slots = [(hp, c, n) for hp in range(2) for c in range(2) for n in range(4)]
        # exp-offload depth per slot: 1 in (0,0,*), 2 in (0,1,*)/(1,0,*),
        # 0 in (1,1,*) where PE is the regional binder and ACT has slack.
        nd11 = int(os.environ.get("K_NDS", "0"))
        nd00 = int(os.environ.get("K_ND0", "1"))
        nds = [nd00] * 4 + [2] * 8 + [nd11] * 4
        ex = {0: {}, 1: {}}

        # ---- lead-in
        if os.environ.get("K_PAIR", "0") == "1":
            emit_qk_pair(ps1p, ("wq", 0, 0), ("wk", 0, 0), fast=True)
        else:
            emit_qk_tile(ps1p, "wq", 0, 0, fast=True)
            emit_qk_tile(ps1p, "wk", 0, 0, fast=True)
        emit_qk_tile(ps1p, "wk", 0, 1, fast=True)

        for g in range(1, 17):
            hp, c, n = slots[g - 1]
            nd = nds[g - 1]
            emit_scd(hp, c, n, ex[hp], nd)
            # first ACT batches before the W@V double so ACT never waits
            emit_scores(hp, c, n, ex[hp], nd, part=0)
            prev = slots[g - 2] if g >= 2 else None
            late_wv = (g == 2 and os.environ.get("K_G2", "0") == "1")
            if prev is not None and not late_wv:
                emit_wv_double(*prev, ex[prev[0]])
            emit_scores(hp, c, n, ex[hp], nd, part=1)
            if prev is not None and late_wv:
                emit_wv_double(*prev, ex[prev[0]])
            if prev is not None and prev[0] == 1 and prev[1] == 1:
                emit_out_pair((4 * prev[2], 4 * prev[2] + 1))
                emit_out_pair((4 * prev[2] + 2, 4 * prev[2] + 3))
            for f in fill.get(g, ()):
                f()
        # drain: last group's two halves interleaved, then the last two pairs
        # with copies split between DVE and the (now idle) ScalarE.
        emit_wv_double(1, 1, 3, ex[1])
        if os.environ.get("K_QUAD", "0") == "1":
            emit_out_pair((12, 13, 14, 15), tail=True)
        else:
            emit_out_pair((12, 13), tail=True)
            emit_out_pair((14, 15), tail=True)


def _build_nc(reps=1):
    nc = bacc.Bacc("TRN2", target_bir_lowering=False, debug=False, num_devices=8)

    aps = (
        nc.dram_tensor("xT", [D, S], BF16, kind="ExternalInput").ap(),
        nc.dram_tensor("wq", [D, JL], BF16, kind="ExternalInput").ap(),
        nc.dram_tensor("wk", [D, JL], BF16, kind="ExternalInput").ap(),
        nc.dram_tensor("wv", [D, JL], BF16, kind="ExternalInput").ap(),
        nc.dram_tensor("wo", [JL, D], BF16, kind="ExternalInput").ap(),
        nc.dram_tensor("c2", [128, S], BF16, kind="ExternalInput").ap(),
        nc.dram_tensor("s2", [128, S], BF16, kind="ExternalInput").ap(),
        nc.dram_tensor("out", [S, D], BF16, kind="ExternalOutput").ap(),
    )

    with (
        tile.TileContext(nc) as tc,
        tc.tile_pool(name="persist", bufs=1) as persist,
        tc.tile_pool(name="rope", bufs=3) as rope,
    ):
        for rep in range(reps):
            _emit_body(nc, tc, persist, rope, aps, rep)

    nc.compile()
    return nc


def _get_nc(reps=1):
    if reps not in _CACHED:
        _CACHED[reps] = _build_nc(reps)
    return _CACHED[reps]


def _host_prep(hidden_states, freqs_cis, Wq, Wk, Wv, Wo):
    bf16 = ml_dtypes.bfloat16
    hs = np.asarray(hidden_states, dtype=np.float32)
    fc = np.asarray(freqs_cis, dtype=np.float32)
    Wq = np.asarray(Wq, dtype=np.float32)
    Wk = np.asarray(Wk, dtype=np.float32)
    Wv = np.asarray(Wv, dtype=np.float32)
    Wo = np.asarray(Wo, dtype=np.float32)

    cos, sin = fc[:, :, 0], fc[:, :, 1]                      # [S, 32]
    i_idx = np.arange(128) % 32
    sign = np.where((np.arange(128) % 64) < 32, -1.0, 1.0).astype(np.float32)
    c2 = np.ascontiguousarray(cos.T[i_idx]).astype(bf16)     # [128, S]
    s2 = np.ascontiguousarray(sin.T[i_idx] * sign[:, None]).astype(bf16)

    xTs = [np.ascontiguousarray(hs[b].T).astype(bf16) for b in range(B)]

    in_maps = []
    for core in range(8):
        b, g = core // 4, core % 4
        perm = []
        for h in range(4 * g, 4 * g + 4):
            perm += [h * 64 + 2 * i for i in range(32)]
            perm += [h * 64 + 2 * i + 1 for i in range(32)]
        perm = np.array(perm)
        jcols = slice(g * JL, (g + 1) * JL)
        in_maps.append({
            "xT": xTs[b],
            "wq": np.ascontiguousarray(Wq[:, perm] * (HD ** -0.5)).astype(bf16),
            "wk": np.ascontiguousarray(Wk[:, perm]).astype(bf16),
            "wv": np.ascontiguousarray(Wv[:, jcols]).astype(bf16),
            "wo": np.ascontiguousarray(Wo[jcols, :]).astype(bf16),
            "c2": c2,
            "s2": s2,
        })
    return in_maps


def kernel(hidden_states, freqs_cis, Wq, Wk, Wv, Wo, _trace=False, _reps=1):
    nc = _get_nc(_reps)
    in_maps = _host_prep(hidden_states, freqs_cis, Wq, Wk, Wv, Wo)
    if _trace:
        try:
            from antenv.axon_hooks import get_axon_ntff_profile_hook  # noqa: F401
        except ImportError:
            _trace = False
    res = run_bass_kernel_spmd(nc, in_maps, core_ids=list(range(8)), trace=_trace)
    outs = [r["out"].astype(np.float32) for r in res.results]
    full = np.zeros((B, S, D), dtype=np.float32)
    for core in range(8):
        full[core // 4] += outs[core]
    if _trace:
        kernel._last_results = res
    return full
